# revision 6
# baseline (speedup 1.0000x reference)
"""Trainium2 Bass kernel for nn_CausalSelfAttention (tensor-parallel over heads, 8 cores).

Contract: kernel(**inputs) takes FULL unsharded numpy inputs and returns the
FULL output [1, 2048, 1024] float32. Internally: shards over 8 NeuronCores
(2 heads each, Wq/Wk/Wv column-sharded, Wo row-sharded), runs one SPMD Bass
program via run_bass_kernel_spmd, and sums the 8 partial Wo products on the
host (the row-parallel unshard).

Compute structure per core (heads 2c, 2c+1):
  - host passes x pre-transposed (xT [D, T] bf16), W pre-rearranged so device
    DMA is contiguous 2KB lines, rotary tables with s_eff folded (rota/rotb),
    and sigw (the rotary half-swap as a permutation matrix for the PE).
  - q projection contraction-outer (overlaps the x HBM load) together with
    the first half of v; k and second half of v contraction-inner per
    512-chunk so the norm+rotary chain starts per chunk.
  - cosine-norm: sumsq via hselw matmul + Ln/Exp rsqrt on ACT; rotary as
    raw*rota + (sigw@raw)*rotb, then * rw (rw is per-(head,pos) so it
    commutes with the half-swap); sq and the rota mul run on GpSimd to
    offload DVE.
  - attention in S^T layout, both heads packed: S^T = k̂T.T @ q̂T with
    tile_position row-split (concurrent 64-contraction pairs), exp on ACT
    (bf16 out), causal handled by (a) column-trimming the diagonal blocks
    (score/exp/mask/AV all skip fully-masked 128-col sub-blocks) and (b)
    one gpsimd affine_select on the partial sub-block. AV col-packs
    [v | ones] per head so the softmax denominator Z accumulates for free.
  - 1/Z via DVE reciprocal_approx_fast on one packed [128,512] tile/chunk
    (~5x cheaper than reciprocal, 18-bit accurate >> bf16).
  - PE queue software-pipelined: scores run depth-2 ahead of AV so the exp
    latency is hidden; Wo matmul mi-groups of chunk c are spread into chunk
    c+1's attention stream so PSUM-copy latency never stalls PE.
Matmul operands are bf16 (fp32 accumulation in PSUM); softmax stats f32.
"""

import functools
import os
import sys
import types

import numpy as np
import ml_dtypes

for _p in ("/opt/trn_rl_repo", "/root/.axon_site/_ro/trn_rl_repo"):
    if os.path.isdir(_p) and _p not in sys.path:
        sys.path.append(_p)

import concourse.bass as bass
import concourse.mybir as mybir
import concourse.tile as tile
from concourse.bass_utils import run_bass_kernel_spmd

F32 = mybir.dt.float32
BF16 = mybir.dt.bfloat16
NPBF16 = ml_dtypes.bfloat16
NCORES = 8
T = 2048
D = 1024
NH = 16
HD = 64
HPC = NH // NCORES   # heads per core
EPC = HPC * HD       # projection cols per core
ATTN_SCALE = 0.12
NT = T // 512
NK = D // 128

LAST = {}


def _register_ntff_hook():
    """Best-effort: register the axon NTFF profile hook if the image's antenv
    lacks axon_hooks (profiling only; compile/run work without it)."""
    try:
        import antenv.axon_hooks  # noqa: F401
        return
    except ImportError:
        pass
    try:
        import trn_agent_boot.trn_boot as tb

        mod = types.ModuleType("antenv.axon_hooks")
        holder = {}
        mod.set_axon_ntff_profile_hook = lambda h: holder.__setitem__("h", h)
        mod.get_axon_ntff_profile_hook = lambda: holder.get("h")
        sys.modules["antenv.axon_hooks"] = mod
        mod.set_axon_ntff_profile_hook(
            tb._ntff_profile_via_ctypes("/opt/axon/libaxon_pjrt.so")
        )
    except Exception:
        pass


def _split_ctrl_waits(nc, k_default=1):
    """The container's walrus build rejects instructions carrying more than one
    semaphore sync-wait; hoist extra waits onto single-wait NoOps that precede
    the instruction on the same engine queue (AND semantics preserved)."""
    n_nops = 0
    for f in nc.m.functions:
        for blk in f.blocks:
            new, changed = [], False
            for inst in list(blk.instructions):
                si = inst.sync_info
                waits = list(si.on_wait) if si is not None else []
                kmax = 1 if isinstance(inst, mybir.InstDrain) else k_default
                if len(waits) > kmax:
                    for k, w in enumerate(waits[:-kmax]):
                        nop = mybir.InstNoOp(name=f"{inst.name}-sw{k}", ins=[], outs=[])
                        nop.engine = inst.engine
                        nop.sync_info = mybir.SyncInfo(on_wait=[w], on_update=[])
                        new.append(nop)
                        n_nops += 1
                    inst.sync_info = mybir.SyncInfo(
                        on_wait=list(waits[-kmax:]), on_update=list(si.on_update)
                    )
                    changed = True
                new.append(inst)
            if changed:
                blk.instructions = new
    return n_nops


def _build_nc():
    nc = bass.Bass("TRN2", target_bir_lowering=False, debug=False, num_devices=NCORES)

    xT_d = nc.dram_tensor("xT", [D, T], BF16, kind="ExternalInput")
    # w[qkv] pre-rearranged on host: w_s[p, 128*i + f] = W[128*i + p, f]
    wq_d = nc.dram_tensor("wq", [128, D], BF16, kind="ExternalInput")
    wk_d = nc.dram_tensor("wk", [128, D], BF16, kind="ExternalInput")
    wv_d = nc.dram_tensor("wv", [128, D], BF16, kind="ExternalInput")
    wo_d = nc.dram_tensor("wo", [EPC, D], BF16, kind="ExternalInput")
    rota_d = nc.dram_tensor("rota", [EPC, T], BF16, kind="ExternalInput")
    rotb_d = nc.dram_tensor("rotb", [EPC, T], BF16, kind="ExternalInput")
    hselw_d = nc.dram_tensor("hselw", [128, 128], BF16, kind="ExternalInput")
    sigw_d = nc.dram_tensor("sigw", [128, 128], BF16, kind="ExternalInput")
    # partials summed on the host in float64; bf16 partials halve the
    # store-DMA tail and stay well inside the accuracy budget
    out_d = nc.dram_tensor("out", [T, D], BF16, kind="ExternalOutput")

    with tile.TileContext(nc) as tc:
        with (
            tc.tile_pool(name="wt", bufs=1) as wt,
            tc.tile_pool(name="xp", bufs=8) as xp,
            tc.tile_pool(name="raw", bufs=1) as rawp,
            tc.tile_pool(name="rot", bufs=1) as rotp,
            tc.tile_pool(name="sm", bufs=2) as smp,
            tc.tile_pool(name="at", bufs=1) as atp,
            tc.tile_pool(name="ps", bufs=2, space="PSUM") as psp,
            tc.tile_pool(name="py", bufs=1, space="PSUM") as pyp,
            tc.tile_pool(name="mm", bufs=2, space="PSUM") as mmp,
        ):
            # ---- constants / weights / x ----
            wq_s = wt.tile([128, D], BF16, tag="wq")
            wk_s = wt.tile([128, D], BF16, tag="wk")
            wv_s = wt.tile([128, D], BF16, tag="wv")
            wo_s = wt.tile([EPC, D], BF16, tag="wo")
            rota = wt.tile([EPC, T], BF16, tag="rota")
            rotb = wt.tile([EPC, T], BF16, tag="rotb")
            hselw = wt.tile([128, 128], BF16, tag="hselw")
            sigw = wt.tile([128, 128], BF16, tag="sigw")
            eps = wt.tile([128, 1], F32, tag="eps")

            nc.sync.dma_start(wq_s[:], wq_d[:])
            nc.sync.dma_start(wv_s[:], wv_d[:])
            xc = []
            for i in range(NK):
                t_ = xp.tile([128, T], BF16, tag="xc", name=f"xc{i}")
                nc.sync.dma_start(t_[:], xT_d[128 * i : 128 * (i + 1), :])
                xc.append(t_)
            nc.sync.dma_start(wk_s[:], wk_d[:])
            nc.sync.dma_start(rota[:], rota_d[:])
            nc.sync.dma_start(rotb[:], rotb_d[:])
            nc.sync.dma_start(hselw[:], hselw_d[:])
            nc.sync.dma_start(sigw[:], sigw_d[:])
            nc.sync.dma_start(wo_s[:], wo_d[:])
            nc.gpsimd.memset(eps[:], 1e-12)

            vext = wt.tile([128, (T // 128) * 256], BF16, tag="vext")
            nc.gpsimd.memset(vext[:], 1.0)
            ident = wt.tile([128, 128], BF16, tag="ident")
            nc.gpsimd.memset(ident[:], 0.0)
            nc.gpsimd.affine_select(
                out=ident[:],
                in_=ident[:],
                compare_op=mybir.AluOpType.not_equal,
                fill=1.0,
                base=0,
                pattern=[[-1, 128]],
                channel_multiplier=1,
            )
            # causal mask addend for diagonal blocks: mask[x, q'] = 0 where
            # q' >= x else -30000; added to scores via an ident-lhsT matmul
            # (keeps the mask off the gpsimd queue and out of the exp->AV
            # dependency chain). Only the first 128 local columns of a
            # diagonal block can be masked, so N=128 covers it.
            mask_s = wt.tile([128, 128], BF16, tag="mask")
            nc.gpsimd.memset(mask_s[:], 0.0)
            nc.gpsimd.affine_select(
                out=mask_s[:],
                in_=mask_s[:],
                compare_op=mybir.AluOpType.is_ge,
                fill=-30000.0,
                base=0,
                pattern=[[1, 128]],
                channel_multiplier=-1,
            )

            q_raw = rawp.tile([128, T], BF16, tag="qraw")
            k_raw = rawp.tile([128, T], BF16, tag="kraw")
            v_raw = rawp.tile([128, T], BF16, tag="vraw")
            qrot = rotp.tile([128, T], BF16, tag="qrot")
            krot = rotp.tile([128, T], BF16, tag="krot")

            # ---- q (outer) + first half of v: overlap the x HBM load ----
            pspair = [
                psp.tile([128, 1024], F32, tag="ps", name=f"pjq{p}") for p in range(2)
            ]
            vacc01 = [
                pyp.tile([128, 512], F32, tag=f"py{n}", name=f"vacc{n}")
                for n in range(2)
            ]
            for i in range(NK):
                for n in range(NT):
                    nc.tensor.matmul(
                        pspair[n // 2][:, 512 * (n % 2) : 512 * (n % 2 + 1)],
                        wq_s[:, 128 * i : 128 * (i + 1)],
                        xc[i][:, 512 * n : 512 * (n + 1)],
                        start=(i == 0),
                        stop=(i == NK - 1),
                    )
                for n in range(2):
                    nc.tensor.matmul(
                        vacc01[n][:],
                        wv_s[:, 128 * i : 128 * (i + 1)],
                        xc[i][:, 512 * n : 512 * (n + 1)],
                        start=(i == 0),
                        stop=(i == NK - 1),
                    )
            for p in range(2):
                nc.scalar.copy(q_raw[:, 1024 * p : 1024 * (p + 1)], pspair[p][:])
            for n in range(2):
                nc.scalar.copy(v_raw[:, 512 * n : 512 * (n + 1)], vacc01[n][:])

            # ---- contraction-inner projection of one 512-chunk ----
            def proj_chunk(w_s, raw, n):
                acc = mmp.tile([128, 512], F32, tag="po", name=f"acc_{raw.name}_{n}")
                for i in range(NK):
                    nc.tensor.matmul(
                        acc[:],
                        w_s[:, 128 * i : 128 * (i + 1)],
                        xc[i][:, 512 * n : 512 * (n + 1)],
                        start=(i == 0),
                        stop=(i == NK - 1),
                    )
                nc.scalar.copy(raw[:, 512 * n : 512 * (n + 1)], acc[:])

            # ---- cosine-norm + rotary for one (tensor, chunk) ----
            # rot = (raw*rota + sigma(raw)*rotb) * rw ; rw = rsqrt(sumsq) is
            # constant within a head so it commutes with the half-swap sigma.
            # sigma done on PE (sigw permutation matmul); sq + rota-mul on
            # GpSimd to offload DVE.
            def norm_chunk(tn, c, raw, rot):
                sl = slice(512 * c, 512 * (c + 1))
                sq = smp.tile([128, 512], BF16, tag="sq", name=f"sq_{tn}{c}")
                nc.gpsimd.tensor_mul(sq[:], raw[:, sl], raw[:, sl])
                ssb = mmp.tile([128, 512], F32, tag="po", name=f"ssb_{tn}{c}")
                nc.tensor.matmul(ssb[:], hselw[:], sq[:], start=True, stop=True)
                lw = smp.tile([128, 512], F32, tag="lw", name=f"lw_{tn}{c}")
                nc.scalar.activation(
                    lw[:], ssb[:], mybir.ActivationFunctionType.Ln, bias=eps[:]
                )
                rw = smp.tile([128, 512], BF16, tag="rw", name=f"rw_{tn}{c}")
                nc.scalar.activation(
                    rw[:], lw[:], mybir.ActivationFunctionType.Exp, scale=-0.5
                )
                swr = psp.tile([128, 512], F32, tag="ps", name=f"swr_{tn}{c}")
                nc.tensor.matmul(swr[:], sigw[:], raw[:, sl], start=True, stop=True)
                t1 = smp.tile([128, 512], BF16, tag="t1", name=f"t1_{tn}{c}")
                nc.gpsimd.tensor_mul(t1[:], raw[:, sl], rota[:, sl])
                t2 = smp.tile([128, 512], BF16, tag="t2", name=f"t2_{tn}{c}")
                nc.vector.tensor_mul(t2[:], swr[:], rotb[:, sl])
                t12 = smp.tile([128, 512], BF16, tag="t12", name=f"t12_{tn}{c}")
                nc.vector.tensor_add(t12[:], t1[:], t2[:])
                nc.vector.tensor_mul(rot[:, sl], t12[:], rw[:])

            # ---- v natural layout: PE transpose + strided scatter into vext ----
            vv = vext[:].rearrange("p (j h f) -> p j h f", h=HPC, f=128)

            def vtp_chunk(j):
                tp_ = mmp.tile([128, 128], BF16, tag="po", name=f"vtp{j}")
                nc.tensor.transpose(tp_[:], v_raw[:, 128 * j : 128 * (j + 1)], ident[:])
                nc.vector.tensor_copy(
                    vv[:, j, :, 0:64],
                    tp_[:].rearrange("p (h f) -> p h f", h=HPC),
                )

            # ---- attention machinery ----
            def scores(c, ps_map, j):
                m = j - 4 * c
                off = 128 * m if m > 0 else 0
                ps2 = psp.tile([128, 1024], F32, tag="ps", name=f"ps2_{c}_{j}")
                for h in range(HPC):
                    hs = slice(64 * h, 64 * (h + 1))
                    nc.tensor.matmul(
                        ps2[:, 512 * h + off : 512 * (h + 1)],
                        krot[hs, 128 * j : 128 * (j + 1)],
                        qrot[hs, 512 * c + off : 512 * (c + 1)],
                        start=True,
                        stop=(m < 0),
                        tile_position=(64 * h, 0),
                    )
                if m >= 0:
                    # diagonal block: accumulate the causal -inf mask onto the
                    # first 128 computed columns of each head's slice
                    for h in range(HPC):
                        nc.tensor.matmul(
                            ps2[:, 512 * h + off : 512 * h + off + 128],
                            ident[:],
                            mask_s[:],
                            start=False,
                            stop=True,
                        )
                ps_map[j] = ps2

            def expj(c, ps_map, pt_map, j):
                m = j - 4 * c
                pt2 = atp.tile([128, 1024], BF16, tag="pt", bufs=4, name=f"pt_{c}_{j}")
                if m >= 1:
                    off = 128 * m
                    pv = ps_map[j].rearrange("p (h q) -> p h q", h=HPC)[:, :, off:]
                    ov = pt2.rearrange("p (h q) -> p h q", h=HPC)[:, :, off:]
                    nc.scalar.activation(
                        ov, pv, mybir.ActivationFunctionType.Exp, scale=ATTN_SCALE
                    )
                else:
                    nc.scalar.activation(
                        pt2[:],
                        ps_map[j][:],
                        mybir.ActivationFunctionType.Exp,
                        scale=ATTN_SCALE,
                    )
                pt_map[j] = pt2

            def av(c, pyh, pt_map, j, nts):
                m = j - 4 * c
                off = 128 * m if m > 0 else 0
                for h in range(HPC):
                    nc.tensor.matmul(
                        pyh[h][:, off:512],
                        vext[:, 256 * j + 128 * h : 256 * j + 128 * (h + 1)],
                        pt_map[j][:, 512 * h + off : 512 * (h + 1)],
                        start=(j == 0),
                        stop=(j == nts - 1),
                    )

            def wo_store(c, mi, yt, wid):
                ost = atp.tile([128, D], BF16, tag="ost", bufs=3, name=f"ost{c}_{mi}")
                for nn in range(2):
                    po = mmp.tile([128, 512], F32, tag="po", name=f"po{c}_{mi}_{nn}")
                    nc.tensor.matmul(
                        po[:],
                        yt[:, 128 * wid : 128 * (wid + 1)],
                        wo_s[:, 512 * nn : 512 * (nn + 1)],
                        start=True,
                        stop=True,
                    )
                    nc.vector.tensor_copy(ost[:, 512 * nn : 512 * (nn + 1)], po[:])
                r0 = 512 * c + 128 * mi
                nc.sync.dma_start(out_d[r0 : r0 + 128, :], ost[:])

            def make_tail(c, pyh):
                # stage A (now): copy y and Z out of PSUM to free the banks
                # for the next chunk's AV accumulation. Stage B (1/Z + mul)
                # and the Wo mi-groups are returned as deferred emitters so
                # they land in the engine queues between the NEXT chunk's
                # exps instead of head-of-line-blocking them.
                ysb = smp.tile([128, 512], BF16, tag="ysb", name=f"ysb{c}")
                zt = smp.tile([128, 512], F32, tag="zt", name=f"zt{c}")
                for h in range(HPC):
                    hs = slice(64 * h, 64 * (h + 1))
                    nc.vector.tensor_copy(ysb[hs, :], pyh[h][0:64, :])
                    nc.vector.tensor_copy(zt[hs, :], pyh[h][64:128, :])
                yt = smp.tile([128, 512], BF16, tag="yt", name=f"yt{c}")

                def stage_b():
                    # 1/Z = exp(-ln Z) on ACT: Ln and Exp share a table set,
                    # and this walrus build rejects the custom-DVE reciprocal.
                    lnz = smp.tile([128, 512], F32, tag="lnz", name=f"lnz{c}")
                    nc.scalar.activation(lnz[:], zt[:], mybir.ActivationFunctionType.Ln)
                    zri = smp.tile([128, 512], BF16, tag="zri", name=f"zri{c}")
                    nc.scalar.activation(
                        zri[:], lnz[:], mybir.ActivationFunctionType.Exp, scale=-1.0
                    )
                    nc.vector.tensor_mul(yt[:], ysb[:], zri[:])

                return [stage_b] + [
                    functools.partial(wo_store, c, mi, yt, mi) for mi in range(4)
                ]

            def attention_chunk(c, pending, last=False):
                nts = 4 * c + 4
                pyh = [
                    pyp.tile([128, 512], F32, tag=f"py{h}", name=f"py{h}_{c}")
                    for h in range(HPC)
                ]
                ps_map, pt_map = {}, {}

                def mi_tail(mi):
                    # last chunk: per-mi pipeline — pyh cols [128mi,128(mi+1))
                    # are final right after AV(4c+mi), so the Wo/store for
                    # those rows overlaps the remaining diagonal blocks.
                    ms = slice(128 * mi, 128 * (mi + 1))
                    ysbm = smp.tile([128, 128], BF16, tag="ysb", name=f"ysbm{mi}")
                    ztm = smp.tile([128, 128], F32, tag="zt", name=f"ztm{mi}")
                    for h in range(HPC):
                        hs = slice(64 * h, 64 * (h + 1))
                        nc.vector.tensor_copy(ysbm[hs, :], pyh[h][0:64, ms])
                        nc.vector.tensor_copy(ztm[hs, :], pyh[h][64:128, ms])
                    lnzm = smp.tile([128, 128], F32, tag="lnz", name=f"lnzm{mi}")
                    nc.scalar.activation(
                        lnzm[:], ztm[:], mybir.ActivationFunctionType.Ln
                    )
                    zrim = smp.tile([128, 128], BF16, tag="zri", name=f"zrim{mi}")
                    nc.scalar.activation(
                        zrim[:], lnzm[:], mybir.ActivationFunctionType.Exp, scale=-1.0
                    )
                    ytm = smp.tile([128, 128], BF16, tag="yt", name=f"ytm{mi}")
                    nc.vector.tensor_mul(ytm[:], ysbm[:], zrim[:])
                    wo_store(c, mi, ytm, 0)

                for j in range(nts):
                    scores(c, ps_map, j)
                    expj(c, ps_map, pt_map, j)
                    if pending and j >= 1 and j % 2 == 1:
                        pending.pop(0)()
                    if j >= 1:
                        av(c, pyh, pt_map, j - 1, nts)
                        if last and j - 1 >= 4 * c:
                            mi_tail(j - 1 - 4 * c)
                av(c, pyh, pt_map, nts - 1, nts)
                if last:
                    mi_tail(3)
                while pending:
                    pending.pop(0)()
                return pyh

            # ---- phase 2 emission: k/v/norms/transposes interleaved with
            # the first attention chunks so attention starts as soon as
            # qrot/krot chunk 0 exist and ACT never waits on the norms ----
            proj_chunk(wk_s, k_raw, 0)
            norm_chunk("k", 0, k_raw, krot)
            proj_chunk(wk_s, k_raw, 1)
            norm_chunk("q", 0, q_raw, qrot)
            for j in range(8):
                vtp_chunk(j)
            pyh0 = attention_chunk(0, [])
            pend = make_tail(0, pyh0)
            proj_chunk(wk_s, k_raw, 2)
            norm_chunk("k", 1, k_raw, krot)
            proj_chunk(wk_s, k_raw, 3)
            norm_chunk("q", 1, q_raw, qrot)
            pyh1 = attention_chunk(1, pend)
            pend = make_tail(1, pyh1)
            proj_chunk(wv_s, v_raw, 2)
            norm_chunk("k", 2, k_raw, krot)
            norm_chunk("q", 2, q_raw, qrot)
            for j in range(8, 12):
                vtp_chunk(j)
            proj_chunk(wv_s, v_raw, 3)
            norm_chunk("k", 3, k_raw, krot)
            norm_chunk("q", 3, q_raw, qrot)
            for j in range(12, 16):
                vtp_chunk(j)
            pyh2 = attention_chunk(2, pend)
            pend = make_tail(2, pyh2)
            attention_chunk(3, pend, last=True)

    return nc


_NC = None
_NC_SPLIT = False


def _host_shards(x, Wq, Wk, Wv, Wo, s_qk):
    x = np.asarray(x, dtype=np.float32)
    Wq = np.asarray(Wq, dtype=np.float32)
    Wk = np.asarray(Wk, dtype=np.float32)
    Wv = np.asarray(Wv, dtype=np.float32)
    Wo = np.asarray(Wo, dtype=np.float32)
    s_qk = np.asarray(s_qk, dtype=np.float32)

    xT = np.ascontiguousarray(x.reshape(T, D).T).astype(NPBF16)

    dim_q = HD // 4
    freq = (1.0 / 1024.0) ** np.linspace(0.0, 1.0, dim_q, dtype=np.float32)
    freq = np.concatenate([freq, np.zeros(dim_q, np.float32)])
    theta = np.arange(T, dtype=np.float32)[:, None] * freq[None, :]
    cosT = np.cos(theta).T.astype(np.float32)
    sinT = np.sin(theta).T.astype(np.float32)
    A64 = np.concatenate([cosT, cosT], 0)
    B64 = np.concatenate([sinT, -sinT], 0)
    s_eff = s_qk * np.float32(np.sqrt(D))

    hselw = np.zeros((128, 128), np.float32)
    for h in range(HPC):
        hselw[64 * h : 64 * (h + 1), 64 * h : 64 * (h + 1)] = 1.0
    hselw = hselw.astype(NPBF16)

    # sigma permutation: sw[m] = raw[sigma(m)], sigma = +-32 within each head.
    # matmul computes out[m] = sum_k lhsT[k, m] rhs[k] -> lhsT[sigma(m), m] = 1
    sigw = np.zeros((128, 128), np.float32)
    for m in range(128):
        h64 = (m // 64) * 64
        lm = m % 64
        sigw[h64 + ((lm + 32) % 64), m] = 1.0
    sigw = sigw.astype(NPBF16)

    def prearrange(Wcols):
        # w_s[p, 128*i + f] = W[128*i + p, f]
        return np.ascontiguousarray(
            Wcols.reshape(NK, 128, EPC).transpose(1, 0, 2).reshape(128, D)
        ).astype(NPBF16)

    in_maps = []
    for c in range(NCORES):
        cols = slice(EPC * c, EPC * (c + 1))
        rota_rows, rotb_rows = [], []
        for h in range(HPC):
            s = s_eff[HPC * c + h]
            s_swap = np.concatenate([s[32:], s[:32]])
            rota_rows.append(s[:, None] * A64)
            rotb_rows.append(s_swap[:, None] * B64)
        in_maps.append(
            {
                "xT": xT,
                "wq": prearrange(Wq[:, cols]),
                "wk": prearrange(Wk[:, cols]),
                "wv": prearrange(Wv[:, cols]),
                "wo": np.ascontiguousarray(Wo[EPC * c : EPC * (c + 1), :]).astype(
                    NPBF16
                ),
                "rota": np.concatenate(rota_rows, 0).astype(NPBF16),
                "rotb": np.concatenate(rotb_rows, 0).astype(NPBF16),
                "hselw": hselw,
                "sigw": sigw,
            }
        )
    return in_maps


def _run_device(in_maps):
    global _NC, _NC_SPLIT
    _register_ntff_hook()
    if _NC is None:
        _NC = _build_nc()
    if not _NC_SPLIT:
        _split_ctrl_waits(_NC)
        _NC_SPLIT = True
    res = run_bass_kernel_spmd(_NC, in_maps, list(range(NCORES)))
    return (
        [np.asarray(r["out"]) for r in res.results],
        res.exec_time_ns,
        res.instructions_and_trace[1] if res.instructions_and_trace else None,
    )


def _worker(in_pkl, out_pkl):
    import pickle

    with open(in_pkl, "rb") as f:
        in_maps = pickle.load(f)
    outs, exec_ns, trace = _run_device(in_maps)
    with open(out_pkl, "wb") as f:
        pickle.dump({"outs": outs, "exec_time_ns": exec_ns, "trace": trace}, f)


def _run_subprocess(in_maps):
    import pickle
    import subprocess
    import tempfile

    d = tempfile.mkdtemp()
    in_pkl = os.path.join(d, "in.pkl")
    out_pkl = os.path.join(d, "out.pkl")
    with open(in_pkl, "wb") as f:
        pickle.dump(in_maps, f)
    here = os.path.dirname(os.path.abspath(__file__))
    code = (
        f"import sys; sys.path.insert(0, {here!r}); "
        f"import kernel; kernel._worker({in_pkl!r}, {out_pkl!r})"
    )
    subprocess.run([sys.executable, "-c", code], check=True, timeout=1800)
    with open(out_pkl, "rb") as f:
        out = pickle.load(f)
    return out["outs"], out["exec_time_ns"], out["trace"]


def _attempt(in_maps, use_subprocess):
    if use_subprocess:
        return _run_subprocess(in_maps)
    return _run_device(in_maps)


def kernel(x, Wq, Wk, Wv, Wo, s_qk):
    in_maps = _host_shards(x, Wq, Wk, Wv, Wo, s_qk)

    def total_of(outs):
        t = np.zeros((T, D), np.float64)
        for o in outs:
            t += o.astype(np.float64)
        return t

    # Run until two executions agree: device runs are deterministic, so a
    # mismatch flags the sporadic silent-corruption failure mode. Crashed
    # runs (NRT unrecoverable) poison this process's PJRT client, so later
    # attempts fall back to fresh subprocesses.
    results = []
    last_exc = None
    sub = False
    for attempt in range(5):
        try:
            outs, exec_ns, trace = _attempt(in_maps, sub)
        except Exception as e:
            last_exc = e
            sub = True
            continue
        t = total_of(outs)
        LAST["exec_time_ns"] = exec_ns
        LAST["trace"] = trace
        for tprev in results:
            denom = max(float(np.abs(tprev).max()), 1e-6)
            if float(np.abs(t - tprev).max()) <= 1e-4 * denom:
                return t.astype(np.float32).reshape(1, T, D)
        results.append(t)
    if results:
        return results[-1].astype(np.float32).reshape(1, T, D)
    raise last_exc


# revision 10
# speedup vs baseline: 1.2651x; 1.2651x over previous
"""Trainium2 Bass kernel for nn_CausalSelfAttention (tensor-parallel over heads, 8 cores).

Contract: kernel(**inputs) takes FULL unsharded numpy inputs and returns the
FULL output [1, 2048, 1024] float32. Internally: shards over 8 NeuronCores
(2 heads each, Wq/Wk/Wv column-sharded, Wo row-sharded), runs one SPMD Bass
program via run_bass_kernel_spmd, and sums the 8 partial Wo products on the
host (the row-parallel unshard).

Compute structure per core (heads 2c, 2c+1):
  - host passes x pre-transposed (xT [D, T] bf16), W pre-rearranged so device
    DMA is contiguous 2KB lines, rotary tables with s_eff folded (rota/rotb),
    and sigw (the rotary half-swap as a permutation matrix for the PE).
  - q projection contraction-outer (overlaps the x HBM load) together with
    the first half of v; k and second half of v contraction-inner per
    512-chunk so the norm+rotary chain starts per chunk.
  - cosine-norm: sumsq via hselw matmul + Ln/Exp rsqrt on ACT; rotary as
    raw*rota + (sigw@raw)*rotb, then * rw (rw is per-(head,pos) so it
    commutes with the half-swap); sq and the rota mul run on GpSimd to
    offload DVE.
  - attention in S^T layout, both heads packed: S^T = k̂T.T @ q̂T with
    tile_position row-split (concurrent 64-contraction pairs), exp on ACT
    (bf16 out), causal handled by (a) column-trimming the diagonal blocks
    (score/exp/mask/AV all skip fully-masked 128-col sub-blocks) and (b)
    one gpsimd affine_select on the partial sub-block. AV col-packs
    [v | ones] per head so the softmax denominator Z accumulates for free.
  - 1/Z via DVE reciprocal_approx_fast on one packed [128,512] tile/chunk
    (~5x cheaper than reciprocal, 18-bit accurate >> bf16).
  - PE queue software-pipelined: scores run depth-2 ahead of AV so the exp
    latency is hidden; Wo matmul mi-groups of chunk c are spread into chunk
    c+1's attention stream so PSUM-copy latency never stalls PE.
Matmul operands are bf16 (fp32 accumulation in PSUM); softmax stats f32.
"""

import functools
import os
import sys
import types

import numpy as np
import ml_dtypes

for _p in ("/opt/trn_rl_repo", "/root/.axon_site/_ro/trn_rl_repo"):
    if os.path.isdir(_p) and _p not in sys.path:
        sys.path.append(_p)

import concourse.bass as bass
import concourse.mybir as mybir
import concourse.tile as tile
from concourse.bass_utils import run_bass_kernel_spmd

F32 = mybir.dt.float32
BF16 = mybir.dt.bfloat16
NPBF16 = ml_dtypes.bfloat16
NCORES = 8
T = 2048
D = 1024
NH = 16
HD = 64
HPC = NH // NCORES   # heads per core
EPC = HPC * HD       # projection cols per core
ATTN_SCALE = 0.12
NT = T // 512
NK = D // 128

LAST = {}


def _register_ntff_hook():
    """Best-effort: register the axon NTFF profile hook if the image's antenv
    lacks axon_hooks (profiling only; compile/run work without it)."""
    try:
        import antenv.axon_hooks  # noqa: F401
        return
    except ImportError:
        pass
    try:
        import trn_agent_boot.trn_boot as tb

        mod = types.ModuleType("antenv.axon_hooks")
        holder = {}
        mod.set_axon_ntff_profile_hook = lambda h: holder.__setitem__("h", h)
        mod.get_axon_ntff_profile_hook = lambda: holder.get("h")
        sys.modules["antenv.axon_hooks"] = mod
        mod.set_axon_ntff_profile_hook(
            tb._ntff_profile_via_ctypes("/opt/axon/libaxon_pjrt.so")
        )
    except Exception:
        pass


def _split_ctrl_waits(nc, k_default=1):
    """The container's walrus build rejects instructions carrying more than one
    semaphore sync-wait; hoist extra waits onto single-wait NoOps that precede
    the instruction on the same engine queue (AND semantics preserved)."""
    n_nops = 0
    for f in nc.m.functions:
        for blk in f.blocks:
            new, changed = [], False
            for inst in list(blk.instructions):
                si = inst.sync_info
                waits = list(si.on_wait) if si is not None else []
                kmax = 1 if isinstance(inst, mybir.InstDrain) else k_default
                if len(waits) > kmax:
                    for k, w in enumerate(waits[:-kmax]):
                        nop = mybir.InstNoOp(name=f"{inst.name}-sw{k}", ins=[], outs=[])
                        nop.engine = inst.engine
                        nop.sync_info = mybir.SyncInfo(on_wait=[w], on_update=[])
                        new.append(nop)
                        n_nops += 1
                    inst.sync_info = mybir.SyncInfo(
                        on_wait=list(waits[-kmax:]), on_update=list(si.on_update)
                    )
                    changed = True
                new.append(inst)
            if changed:
                blk.instructions = new
    return n_nops


def _build_nc():
    nc = bass.Bass("TRN2", target_bir_lowering=False, debug=False, num_devices=NCORES)

    xT_d = nc.dram_tensor("xT", [D, T], BF16, kind="ExternalInput")
    # w[qkv] pre-rearranged on host: w_s[p, 128*i + f] = W[128*i + p, f]
    wq_d = nc.dram_tensor("wq", [128, D], BF16, kind="ExternalInput")
    wk_d = nc.dram_tensor("wk", [128, D], BF16, kind="ExternalInput")
    wv_d = nc.dram_tensor("wv", [128, D], BF16, kind="ExternalInput")
    wo_d = nc.dram_tensor("wo", [EPC, D], BF16, kind="ExternalInput")
    rota_d = nc.dram_tensor("rota", [EPC, T], BF16, kind="ExternalInput")
    rotb_d = nc.dram_tensor("rotb", [EPC, T], BF16, kind="ExternalInput")
    hselw_d = nc.dram_tensor("hselw", [128, 128], BF16, kind="ExternalInput")
    sigw_d = nc.dram_tensor("sigw", [128, 128], BF16, kind="ExternalInput")
    # partials summed on the host in float64; bf16 partials halve the
    # store-DMA tail and stay well inside the accuracy budget
    out_d = nc.dram_tensor("out", [T, D], BF16, kind="ExternalOutput")

    with tile.TileContext(nc) as tc:
        with (
            tc.tile_pool(name="wt", bufs=1) as wt,
            tc.tile_pool(name="xp", bufs=8) as xp,
            tc.tile_pool(name="raw", bufs=1) as rawp,
            tc.tile_pool(name="rot", bufs=1) as rotp,
            tc.tile_pool(name="sm", bufs=2) as smp,
            tc.tile_pool(name="at", bufs=1) as atp,
            tc.tile_pool(name="ps", bufs=2, space="PSUM") as psp,
            tc.tile_pool(name="py", bufs=1, space="PSUM") as pyp,
            tc.tile_pool(name="mm", bufs=2, space="PSUM") as mmp,
        ):
            # ---- constants / weights / x ----
            wq_s = wt.tile([128, D], BF16, tag="wq")
            wk_s = wt.tile([128, D], BF16, tag="wk")
            wv_s = wt.tile([128, D], BF16, tag="wv")
            wo_s = wt.tile([EPC, D], BF16, tag="wo")
            rota = wt.tile([EPC, T], BF16, tag="rota")
            rotb = wt.tile([EPC, T], BF16, tag="rotb")
            hselw = wt.tile([128, 128], BF16, tag="hselw")
            sigw = wt.tile([128, 128], BF16, tag="sigw")
            eps = wt.tile([128, 1], F32, tag="eps")

            nc.sync.dma_start(wq_s[:], wq_d[:])
            nc.sync.dma_start(wv_s[:], wv_d[:])
            xc = []
            for i in range(NK):
                t_ = xp.tile([128, T], BF16, tag="xc", name=f"xc{i}")
                nc.sync.dma_start(t_[:], xT_d[128 * i : 128 * (i + 1), :])
                xc.append(t_)
            nc.sync.dma_start(wk_s[:], wk_d[:])
            nc.sync.dma_start(rota[:], rota_d[:])
            nc.sync.dma_start(rotb[:], rotb_d[:])
            nc.sync.dma_start(hselw[:], hselw_d[:])
            nc.sync.dma_start(sigw[:], sigw_d[:])
            nc.sync.dma_start(wo_s[:], wo_d[:])
            nc.gpsimd.memset(eps[:], 1e-12)

            vext = wt.tile([128, (T // 128) * 256], BF16, tag="vext")
            nc.gpsimd.memset(vext[:], 1.0)
            ident = wt.tile([128, 128], BF16, tag="ident")
            nc.gpsimd.memset(ident[:], 0.0)
            nc.gpsimd.affine_select(
                out=ident[:],
                in_=ident[:],
                compare_op=mybir.AluOpType.not_equal,
                fill=1.0,
                base=0,
                pattern=[[-1, 128]],
                channel_multiplier=1,
            )


            q_raw = rawp.tile([128, T], BF16, tag="qraw")
            k_raw = rawp.tile([128, T], BF16, tag="kraw")
            v_raw = rawp.tile([128, T], BF16, tag="vraw")
            qrot = rotp.tile([128, T], BF16, tag="qrot")
            krot = rotp.tile([128, T], BF16, tag="krot")

            # ---- q (outer) + first half of v: overlap the x HBM load ----
            pspair = [
                psp.tile([128, 1024], F32, tag="ps", name=f"pjq{p}") for p in range(2)
            ]
            vacc01 = [
                pyp.tile([128, 512], F32, tag=f"py{n}", name=f"vacc{n}")
                for n in range(2)
            ]
            for i in range(NK):
                for n in range(NT):
                    nc.tensor.matmul(
                        pspair[n // 2][:, 512 * (n % 2) : 512 * (n % 2 + 1)],
                        wq_s[:, 128 * i : 128 * (i + 1)],
                        xc[i][:, 512 * n : 512 * (n + 1)],
                        start=(i == 0),
                        stop=(i == NK - 1),
                    )
                for n in range(2):
                    nc.tensor.matmul(
                        vacc01[n][:],
                        wv_s[:, 128 * i : 128 * (i + 1)],
                        xc[i][:, 512 * n : 512 * (n + 1)],
                        start=(i == 0),
                        stop=(i == NK - 1),
                    )
            for p in range(2):
                nc.scalar.copy(q_raw[:, 1024 * p : 1024 * (p + 1)], pspair[p][:])
            for n in range(2):
                nc.scalar.copy(v_raw[:, 512 * n : 512 * (n + 1)], vacc01[n][:])

            # ---- contraction-inner projection of one 512-chunk ----
            def proj_chunk(w_s, raw, n):
                acc = mmp.tile([128, 512], F32, tag="po", name=f"acc_{raw.name}_{n}")
                for i in range(NK):
                    nc.tensor.matmul(
                        acc[:],
                        w_s[:, 128 * i : 128 * (i + 1)],
                        xc[i][:, 512 * n : 512 * (n + 1)],
                        start=(i == 0),
                        stop=(i == NK - 1),
                    )
                nc.scalar.copy(raw[:, 512 * n : 512 * (n + 1)], acc[:])

            # ---- cosine-norm + rotary for one (tensor, chunk) ----
            # rot = (raw*rota + sigma(raw)*rotb) * rw ; rw = rsqrt(sumsq) is
            # constant within a head so it commutes with the half-swap sigma.
            # sigma done on PE (sigw permutation matmul); sq + rota-mul on
            # GpSimd to offload DVE.
            def norm_chunk(tn, c, raw, rot):
                sl = slice(512 * c, 512 * (c + 1))
                sq = smp.tile([128, 512], BF16, tag="sq", name=f"sq_{tn}{c}")
                nc.vector.tensor_mul(sq[:], raw[:, sl], raw[:, sl])
                ssb = mmp.tile([128, 512], F32, tag="po", name=f"ssb_{tn}{c}")
                nc.tensor.matmul(ssb[:], hselw[:], sq[:], start=True, stop=True)
                lw = smp.tile([128, 512], F32, tag="lw", name=f"lw_{tn}{c}")
                nc.scalar.activation(
                    lw[:], ssb[:], mybir.ActivationFunctionType.Ln, bias=eps[:]
                )
                rw = smp.tile([128, 512], BF16, tag="rw", name=f"rw_{tn}{c}")
                nc.scalar.activation(
                    rw[:], lw[:], mybir.ActivationFunctionType.Exp, scale=-0.5
                )
                swr = psp.tile([128, 512], F32, tag="ps", name=f"swr_{tn}{c}")
                nc.tensor.matmul(swr[:], sigw[:], raw[:, sl], start=True, stop=True)
                t1 = smp.tile([128, 512], BF16, tag="t1", name=f"t1_{tn}{c}")
                nc.vector.tensor_mul(t1[:], raw[:, sl], rota[:, sl])
                t2 = smp.tile([128, 512], BF16, tag="t2", name=f"t2_{tn}{c}")
                nc.vector.tensor_mul(t2[:], swr[:], rotb[:, sl])
                t12 = smp.tile([128, 512], BF16, tag="t12", name=f"t12_{tn}{c}")
                nc.vector.tensor_add(t12[:], t1[:], t2[:])
                nc.vector.tensor_mul(rot[:, sl], t12[:], rw[:])

            # ---- v natural layout: PE transpose + strided scatter into vext ----
            vv = vext[:].rearrange("p (j h f) -> p j h f", h=HPC, f=128)

            def vtp_chunk(j):
                tp_ = mmp.tile([128, 128], BF16, tag="po", name=f"vtp{j}")
                nc.tensor.transpose(tp_[:], v_raw[:, 128 * j : 128 * (j + 1)], ident[:])
                nc.vector.tensor_copy(
                    vv[:, j, :, 0:64],
                    tp_[:].rearrange("p (h f) -> p h f", h=HPC),
                )

            # ---- attention machinery ----
            is_ge = mybir.AluOpType.is_ge

            def scores(c, ps_map, j):
                m = j - 4 * c
                off = 128 * m if m > 0 else 0
                ps2 = psp.tile([128, 1024], F32, tag="ps", name=f"ps2_{c}_{j}")
                for h in range(HPC):
                    hs = slice(64 * h, 64 * (h + 1))
                    nc.tensor.matmul(
                        ps2[:, 512 * h + off : 512 * (h + 1)],
                        krot[hs, 128 * j : 128 * (j + 1)],
                        qrot[hs, 512 * c + off : 512 * (c + 1)],
                        start=True,
                        stop=True,
                        tile_position=(64 * h, 0),
                    )
                ps_map[j] = ps2

            def expj(c, ps_map, pt_map, j):
                m = j - 4 * c
                pt2 = atp.tile([128, 1024], BF16, tag="pt", bufs=4, name=f"pt_{c}_{j}")
                if m >= 1:
                    off = 128 * m
                    qn = 512 - off
                    pv = ps_map[j].rearrange("p (h q) -> p h q", h=HPC)[:, :, off:]
                    ov = pt2.rearrange("p (h q) -> p h q", h=HPC)[:, :, off:]
                    nc.scalar.activation(
                        ov, pv, mybir.ActivationFunctionType.Exp, scale=ATTN_SCALE
                    )
                    nc.gpsimd.affine_select(
                        out=ov,
                        in_=ov,
                        compare_op=is_ge,
                        fill=0.0,
                        base=0,
                        pattern=[[0, HPC], [1, qn]],
                        channel_multiplier=-1,
                    )
                else:
                    nc.scalar.activation(
                        pt2[:],
                        ps_map[j][:],
                        mybir.ActivationFunctionType.Exp,
                        scale=ATTN_SCALE,
                    )
                    if m == 0:
                        nc.gpsimd.affine_select(
                            out=pt2[:],
                            in_=pt2[:],
                            compare_op=is_ge,
                            fill=0.0,
                            base=0,
                            pattern=[[0, HPC], [1, 512]],
                            channel_multiplier=-1,
                        )
                pt_map[j] = pt2

            def av(c, pyh, pt_map, j, nts):
                m = j - 4 * c
                off = 128 * m if m > 0 else 0
                for h in range(HPC):
                    nc.tensor.matmul(
                        pyh[h][:, off:512],
                        vext[:, 256 * j + 128 * h : 256 * j + 128 * (h + 1)],
                        pt_map[j][:, 512 * h + off : 512 * (h + 1)],
                        start=(j == 0),
                        stop=(j == nts - 1),
                    )

            def wo_store(c, mi, yt, wid):
                ost = atp.tile([128, D], BF16, tag="ost", bufs=3, name=f"ost{c}_{mi}")
                for nn in range(2):
                    po = mmp.tile([128, 512], F32, tag="po", name=f"po{c}_{mi}_{nn}")
                    nc.tensor.matmul(
                        po[:],
                        yt[:, 128 * wid : 128 * (wid + 1)],
                        wo_s[:, 512 * nn : 512 * (nn + 1)],
                        start=True,
                        stop=True,
                    )
                    nc.vector.tensor_copy(ost[:, 512 * nn : 512 * (nn + 1)], po[:])
                r0 = 512 * c + 128 * mi
                nc.sync.dma_start(out_d[r0 : r0 + 128, :], ost[:])

            def make_tail(c, pyh):
                # stage A (now): copy y and Z out of PSUM to free the banks
                # for the next chunk's AV accumulation. Stage B (1/Z + mul)
                # and the Wo mi-groups are returned as deferred emitters so
                # they land in the engine queues between the NEXT chunk's
                # exps instead of head-of-line-blocking them.
                ysb = smp.tile([128, 512], BF16, tag="ysb", name=f"ysb{c}")
                zt = smp.tile([128, 512], F32, tag="zt", name=f"zt{c}")
                for h in range(HPC):
                    hs = slice(64 * h, 64 * (h + 1))
                    nc.vector.tensor_copy(ysb[hs, :], pyh[h][0:64, :])
                    nc.vector.tensor_copy(zt[hs, :], pyh[h][64:128, :])
                yt = smp.tile([128, 512], BF16, tag="yt", name=f"yt{c}")

                def stage_b():
                    # 1/Z = exp(-ln Z) on ACT: Ln and Exp share a table set,
                    # and this walrus build rejects the custom-DVE reciprocal.
                    lnz = smp.tile([128, 512], F32, tag="lnz", name=f"lnz{c}")
                    nc.scalar.activation(lnz[:], zt[:], mybir.ActivationFunctionType.Ln)
                    zri = smp.tile([128, 512], BF16, tag="zri", name=f"zri{c}")
                    nc.scalar.activation(
                        zri[:], lnz[:], mybir.ActivationFunctionType.Exp, scale=-1.0
                    )
                    nc.vector.tensor_mul(yt[:], ysb[:], zri[:])

                return [stage_b] + [
                    functools.partial(wo_store, c, mi, yt, mi) for mi in range(4)
                ]

            def attention_chunk(c, pending, last=False):
                nts = 4 * c + 4
                pyh = [
                    pyp.tile([128, 512], F32, tag=f"py{h}", name=f"py{h}_{c}")
                    for h in range(HPC)
                ]
                ps_map, pt_map = {}, {}

                def mi_tail(mi):
                    # last chunk: per-mi pipeline — pyh cols [128mi,128(mi+1))
                    # are final right after AV(4c+mi), so the Wo/store for
                    # those rows overlaps the remaining diagonal blocks.
                    ms = slice(128 * mi, 128 * (mi + 1))
                    ysbm = smp.tile([128, 128], BF16, tag="ysb", name=f"ysbm{mi}")
                    ztm = smp.tile([128, 128], F32, tag="zt", name=f"ztm{mi}")
                    for h in range(HPC):
                        hs = slice(64 * h, 64 * (h + 1))
                        nc.vector.tensor_copy(ysbm[hs, :], pyh[h][0:64, ms])
                        nc.vector.tensor_copy(ztm[hs, :], pyh[h][64:128, ms])
                    lnzm = smp.tile([128, 128], F32, tag="lnz", name=f"lnzm{mi}")
                    nc.scalar.activation(
                        lnzm[:], ztm[:], mybir.ActivationFunctionType.Ln
                    )
                    zrim = smp.tile([128, 128], BF16, tag="zri", name=f"zrim{mi}")
                    nc.scalar.activation(
                        zrim[:], lnzm[:], mybir.ActivationFunctionType.Exp, scale=-1.0
                    )
                    ytm = smp.tile([128, 128], BF16, tag="yt", name=f"ytm{mi}")
                    nc.vector.tensor_mul(ytm[:], ysbm[:], zrim[:])
                    wo_store(c, mi, ytm, 0)

                for j in range(nts):
                    scores(c, ps_map, j)
                    expj(c, ps_map, pt_map, j)
                    if pending and j >= 1 and j % 2 == 1:
                        pending.pop(0)()
                    if j >= 1:
                        av(c, pyh, pt_map, j - 1, nts)
                        if last and j - 1 >= 4 * c:
                            mi_tail(j - 1 - 4 * c)
                av(c, pyh, pt_map, nts - 1, nts)
                if last:
                    mi_tail(3)
                while pending:
                    pending.pop(0)()
                return pyh

            # ---- phase 2 emission: k/v/norms/transposes interleaved with
            # the first attention chunks so attention starts as soon as
            # qrot/krot chunk 0 exist and ACT never waits on the norms ----
            proj_chunk(wk_s, k_raw, 0)
            norm_chunk("k", 0, k_raw, krot)
            proj_chunk(wk_s, k_raw, 1)
            norm_chunk("q", 0, q_raw, qrot)
            for j in range(8):
                vtp_chunk(j)
            pyh0 = attention_chunk(0, [])
            pend = make_tail(0, pyh0)
            proj_chunk(wk_s, k_raw, 2)
            norm_chunk("k", 1, k_raw, krot)
            proj_chunk(wk_s, k_raw, 3)
            norm_chunk("q", 1, q_raw, qrot)
            pyh1 = attention_chunk(1, pend)
            pend = make_tail(1, pyh1)
            proj_chunk(wv_s, v_raw, 2)
            norm_chunk("k", 2, k_raw, krot)
            norm_chunk("q", 2, q_raw, qrot)
            for j in range(8, 12):
                vtp_chunk(j)
            proj_chunk(wv_s, v_raw, 3)
            norm_chunk("k", 3, k_raw, krot)
            norm_chunk("q", 3, q_raw, qrot)
            for j in range(12, 16):
                vtp_chunk(j)
            pyh2 = attention_chunk(2, pend)
            pend = make_tail(2, pyh2)
            attention_chunk(3, pend, last=True)

    return nc


_NC = None
_NC_SPLIT = False


def _host_shards(x, Wq, Wk, Wv, Wo, s_qk):
    x = np.asarray(x, dtype=np.float32)
    Wq = np.asarray(Wq, dtype=np.float32)
    Wk = np.asarray(Wk, dtype=np.float32)
    Wv = np.asarray(Wv, dtype=np.float32)
    Wo = np.asarray(Wo, dtype=np.float32)
    s_qk = np.asarray(s_qk, dtype=np.float32)

    xT = np.ascontiguousarray(x.reshape(T, D).T).astype(NPBF16)

    dim_q = HD // 4
    freq = (1.0 / 1024.0) ** np.linspace(0.0, 1.0, dim_q, dtype=np.float32)
    freq = np.concatenate([freq, np.zeros(dim_q, np.float32)])
    theta = np.arange(T, dtype=np.float32)[:, None] * freq[None, :]
    cosT = np.cos(theta).T.astype(np.float32)
    sinT = np.sin(theta).T.astype(np.float32)
    A64 = np.concatenate([cosT, cosT], 0)
    B64 = np.concatenate([sinT, -sinT], 0)
    s_eff = s_qk * np.float32(np.sqrt(D))

    hselw = np.zeros((128, 128), np.float32)
    for h in range(HPC):
        hselw[64 * h : 64 * (h + 1), 64 * h : 64 * (h + 1)] = 1.0
    hselw = hselw.astype(NPBF16)

    # sigma permutation: sw[m] = raw[sigma(m)], sigma = +-32 within each head.
    # matmul computes out[m] = sum_k lhsT[k, m] rhs[k] -> lhsT[sigma(m), m] = 1
    sigw = np.zeros((128, 128), np.float32)
    for m in range(128):
        h64 = (m // 64) * 64
        lm = m % 64
        sigw[h64 + ((lm + 32) % 64), m] = 1.0
    sigw = sigw.astype(NPBF16)

    def prearrange(Wcols):
        # w_s[p, 128*i + f] = W[128*i + p, f]
        return np.ascontiguousarray(
            Wcols.reshape(NK, 128, EPC).transpose(1, 0, 2).reshape(128, D)
        ).astype(NPBF16)

    in_maps = []
    for c in range(NCORES):
        cols = slice(EPC * c, EPC * (c + 1))
        rota_rows, rotb_rows = [], []
        for h in range(HPC):
            s = s_eff[HPC * c + h]
            s_swap = np.concatenate([s[32:], s[:32]])
            rota_rows.append(s[:, None] * A64)
            rotb_rows.append(s_swap[:, None] * B64)
        in_maps.append(
            {
                "xT": xT,
                "wq": prearrange(Wq[:, cols]),
                "wk": prearrange(Wk[:, cols]),
                "wv": prearrange(Wv[:, cols]),
                "wo": np.ascontiguousarray(Wo[EPC * c : EPC * (c + 1), :]).astype(
                    NPBF16
                ),
                "rota": np.concatenate(rota_rows, 0).astype(NPBF16),
                "rotb": np.concatenate(rotb_rows, 0).astype(NPBF16),
                "hselw": hselw,
                "sigw": sigw,
            }
        )
    return in_maps


def _run_device(in_maps):
    global _NC, _NC_SPLIT
    _register_ntff_hook()
    if _NC is None:
        _NC = _build_nc()
    if not _NC_SPLIT:
        _split_ctrl_waits(_NC)
        _NC_SPLIT = True
    res = run_bass_kernel_spmd(_NC, in_maps, list(range(NCORES)))
    return (
        [np.asarray(r["out"]) for r in res.results],
        res.exec_time_ns,
        res.instructions_and_trace[1] if res.instructions_and_trace else None,
    )


def _worker(in_pkl, out_pkl):
    import pickle

    with open(in_pkl, "rb") as f:
        in_maps = pickle.load(f)
    outs, exec_ns, trace = _run_device(in_maps)
    with open(out_pkl, "wb") as f:
        pickle.dump({"outs": outs, "exec_time_ns": exec_ns, "trace": trace}, f)


def _run_subprocess(in_maps):
    import pickle
    import subprocess
    import tempfile

    d = tempfile.mkdtemp()
    in_pkl = os.path.join(d, "in.pkl")
    out_pkl = os.path.join(d, "out.pkl")
    with open(in_pkl, "wb") as f:
        pickle.dump(in_maps, f)
    here = os.path.dirname(os.path.abspath(__file__))
    code = (
        f"import sys; sys.path.insert(0, {here!r}); "
        f"import kernel; kernel._worker({in_pkl!r}, {out_pkl!r})"
    )
    subprocess.run([sys.executable, "-c", code], check=True, timeout=1800)
    with open(out_pkl, "rb") as f:
        out = pickle.load(f)
    return out["outs"], out["exec_time_ns"], out["trace"]


def _attempt(in_maps, use_subprocess):
    if use_subprocess:
        return _run_subprocess(in_maps)
    return _run_device(in_maps)


def kernel(x, Wq, Wk, Wv, Wo, s_qk):
    in_maps = _host_shards(x, Wq, Wk, Wv, Wo, s_qk)

    def total_of(outs):
        t = np.zeros((T, D), np.float64)
        for o in outs:
            t += o.astype(np.float64)
        return t

    # Run until two executions agree: device runs are deterministic, so a
    # mismatch flags the sporadic silent-corruption failure mode. Crashed
    # runs (NRT unrecoverable) poison this process's PJRT client, so later
    # attempts fall back to fresh subprocesses.
    results = []
    last_exc = None
    sub = False
    for attempt in range(5):
        try:
            outs, exec_ns, trace = _attempt(in_maps, sub)
        except Exception as e:
            last_exc = e
            sub = True
            continue
        t = total_of(outs)
        LAST["exec_time_ns"] = exec_ns
        LAST["trace"] = trace
        for tprev in results:
            denom = max(float(np.abs(tprev).max()), 1e-6)
            if float(np.abs(t - tprev).max()) <= 1e-4 * denom:
                return t.astype(np.float32).reshape(1, T, D)
        results.append(t)
    if results:
        return results[-1].astype(np.float32).reshape(1, T, D)
    raise last_exc
